# revision 17
# baseline (speedup 1.0000x reference)
"""Trainium2 Bass kernel for nn_A_9835475108252 (dense_transformer).

Sharding: pure data parallel — batch 64 split as 8 elements per NeuronCore,
all weights replicated. Inside each core, per batch element:
  branch a/b = token-axis self-attention (TSA) + feature-axis SA (FSA),
  projections to 100 dims, G = aT^T@aF outer-product-ish [100,100],
  vec(G) @ {a,b}ft_w [10000,1000]; plus a tiny 2-layer head on CLS tokens.

Layout strategy: every matmul is expressed as out = lhsT^T @ rhs with the
contraction dim on partitions; the host uploads both [l,d] and [d,l] copies
of the activations so the device never transposes. Softmax denominators are
computed with PE matmuls against a ones vector, oriented so the 1/sum scale
folds into a per-partition tensor_scalar at the next eviction.
"""
import math
import numpy as np
import ml_dtypes

import concourse.bass as bass
from concourse import bacc
import concourse.mybir as mybir
import concourse.tile as tile
from concourse.bass_utils import run_bass_kernel_spmd

P = 128
F32 = mybir.dt.float32
F32R = mybir.dt.float32r
BF16 = mybir.dt.bfloat16
FP8 = mybir.dt.float8e4
EXP = mybir.ActivationFunctionType.Exp

B, NCORE, BPC, H = 64, 8, 8, 768
LA, LB = 272, 256
HT = H // P  # 6
LT_A = [(0, 128), (128, 128), (256, 16)]
LT_B = [(0, 128), (128, 128)]

_CACHE: dict = {}


def _acc_mm(nc, ps, pairs):
    n = len(pairs)
    for i, (lt, rt) in enumerate(pairs):
        nc.tensor.matmul(ps, lt, rt, start=(i == 0), stop=(i == n - 1))


def _load_acts(nc, acts, e, L, ltiles, xt_dram, xf_dram):
    xt = acts.tile([P, 3, H], BF16, tag="xt", name="xt")  # [l-part, lo, d]
    for i, (off, sz) in enumerate(ltiles):
        nc.sync.dma_start(xt[:sz, i, :], xt_dram[e, off:off + sz, :])
    xf = acts.tile([P, HT, LA], BF16, tag="xf", name="xf")  # [d-part, do, l]
    nc.sync.dma_start(xf[:, :, :L], xf_dram[e])
    return xt, xf


def _emit_branch(nc, pools, e, L, ltiles, xt_dram, xf_dram, qkv_sb, fw_sb,
                 tw_sb, fw2_sb, gt_sb, scale_t, scale_f, preloaded=None):
    work, acts, pp = pools["work"], pools["acts"], pools["ps"]
    ones = pools["ones"]

    # ---- activations -------------------------------------------------
    xt, xf = preloaded if preloaded is not None else _load_acts(
        nc, acts, e, L, ltiles, xt_dram, xf_dram)

    # ---- TSA ----------------------------------------------------------
    # uf [d_out-part, do, l] = (x @ (Wq Wk^T))^T  (Q/K folded on host)
    uf = work.tile([P, HT, LA], BF16, tag="qf", name="uf")
    for do in range(HT):
        ps = pp.tile([P, 512], F32, tag="ps", name="ps_q")[:, :L]
        _acc_mm(nc, ps, [(qkv_sb[:, 0, ki, do * P:(do + 1) * P],
                          xf[:, ki, :L]) for ki in range(HT)])
        nc.vector.tensor_copy(out=uf[:, do, :L], in_=ps)
    # v [l-part, lo, d] = x @ Wv
    v = work.tile([P, 3, H], BF16, tag="v", name="v")
    for lt, (off, sz) in enumerate(ltiles):
        for c0 in (0, 384):
            ps = pp.tile([P, 512], F32, tag="ps", name="ps_v")[:sz, :384]
            _acc_mm(nc, ps, [(xf[:, ki, off:off + sz],
                              qkv_sb[:, 1, ki, c0:c0 + 384]) for ki in range(HT)])
            nc.vector.tensor_copy(out=v[:sz, lt, c0:c0 + 384], in_=ps)
    # est [m-part, mo, l] = exp(scale * S^T),  S = q @ k^T
    est = work.tile([P, 3, LA], BF16, tag="est", name="est")
    for mt, (moff, msz) in enumerate(ltiles):
        ps = pp.tile([P, 512], F32, tag="ps", name="ps_st")[:msz, :L]
        _acc_mm(nc, ps, [(xf[:, ki, moff:moff + msz], uf[:, ki, :L])
                         for ki in range(HT)])
        nc.scalar.activation(out=est[:msz, mt, :L], in_=ps, func=EXP,
                             scale=scale_t)
    # sinv [l-part, lo, 1] = 1 / colsum(est)
    sinv = work.tile([P, 3, 1], F32, tag="sinv", name="sinv")
    for lt, (loff, lsz) in enumerate(ltiles):
        ps = pp.tile([P, 512], F32, tag="ps", name="ps_si")[:lsz, :1]
        _acc_mm(nc, ps, [(est[:msz, mt, loff:loff + lsz], ones[:msz, :])
                         for mt, (moff, msz) in enumerate(ltiles)])
        nc.vector.reciprocal(out=sinv[:lsz, lt, :], in_=ps)
    # oa [d-part, do, l] = v^T @ est   (unnormalized attention out, F-layout)
    oa = work.tile([P, HT, LA], BF16, tag="oa", name="oa")
    for do in range(HT):
        ps = pp.tile([P, 512], F32, tag="ps", name="ps_oa")[:, :L]
        _acc_mm(nc, ps, [(v[:msz, mt, do * P:(do + 1) * P], est[:msz, mt, :L])
                         for mt, (moff, msz) in enumerate(ltiles)])
        nc.vector.tensor_copy(out=oa[:, do, :L], in_=ps)
    # aT [l-part, lo, 100] = (aTSA @ atW) * sinv  (softmax norm folded here)
    aT = work.tile([P, 3, 100], BF16, tag="aT", name="aT")
    for lt, (loff, lsz) in enumerate(ltiles):
        ps = pp.tile([P, 512], F32, tag="ps", name="ps_at")[:lsz, :100]
        _acc_mm(nc, ps, [(oa[:, ki, loff:loff + lsz], tw_sb[:, ki, :])
                         for ki in range(HT)])
        nc.vector.tensor_scalar_mul(out=aT[:lsz, lt, :], in0=ps,
                                    scalar1=sinv[:lsz, lt, :])

    # ---- FSA ----------------------------------------------------------
    # wt [t-part, to, d] = (Wfq Wfk^T)^T @ x   (Q/K folded on host)
    wt = work.tile([P, 3, H], BF16, tag="qft", name="wt")
    for tt, (toff, tsz) in enumerate(ltiles):
        for c0 in (0, 384):
            ps = pp.tile([P, 512], F32, tag="ps", name="ps_qf")[:tsz, :384]
            _acc_mm(nc, ps, [(fw_sb[:ksz, 0, ki, toff:toff + tsz],
                              xt[:ksz, ki, c0:c0 + 384])
                             for ki, (koff, ksz) in enumerate(ltiles)])
            nc.vector.tensor_copy(out=wt[:tsz, tt, c0:c0 + 384], in_=ps)
    # vf [e-part, eo, t (+ ones col at L)] = x^T @ Wfv
    vf = work.tile([P, HT, LA + 1], BF16, tag="vf", name="vf")
    nc.vector.memset(vf[:, :, L:L + 1], 1.0)
    for eo in range(HT):
        ps = pp.tile([P, 512], F32, tag="ps", name="ps_vf")[:, :L]
        _acc_mm(nc, ps, [(xt[:ksz, ki, eo * P:(eo + 1) * P],
                          fw_sb[:ksz, 1, ki, :L])
                         for ki, (koff, ksz) in enumerate(ltiles)])
        nc.vector.tensor_copy(out=vf[:, eo, :L], in_=ps)
    # esf [e-part, eo, d] = exp(scale * Sf^T)
    esf = work.tile([P, HT, H], BF16, tag="esf", name="esf")
    for eo in range(HT):
        for c0 in (0, 384):
            ps = pp.tile([P, 512], F32, tag="ps", name="ps_sf")[:, :384]
            _acc_mm(nc, ps, [(xt[:tsz, ki, eo * P:(eo + 1) * P],
                              wt[:tsz, ki, c0:c0 + 384])
                             for ki, (toff, tsz) in enumerate(ltiles)])
            nc.scalar.activation(out=esf[:, eo, c0:c0 + 384], in_=ps, func=EXP,
                                 scale=scale_f)
    # of [d-part, do, t] = (Pf @ vf); the vf ones-column makes psum col L
    # hold colsum(esf) = the softmax denominator, normalized at eviction
    sfinv = work.tile([P, HT, 1], F32, tag="sfinv", name="sfinv")
    of = work.tile([P, HT, LA], BF16, tag="of", name="of")
    for do in range(HT):
        ps = pp.tile([P, 512], F32, tag="ps", name="ps_of")[:, :L + 1]
        _acc_mm(nc, ps, [(esf[:, ki, do * P:(do + 1) * P], vf[:, ki, :L + 1])
                         for ki in range(HT)])
        nc.vector.reciprocal(out=sfinv[:, do, :], in_=ps[:, L:L + 1])
        nc.vector.tensor_scalar_mul(out=of[:, do, :L], in0=ps[:, :L],
                                    scalar1=sfinv[:, do, :])
    # aF [l-part, lo, 100] = aFSA @ afW
    aF = work.tile([P, 3, 100], BF16, tag="aF", name="aF")
    for lt, (loff, lsz) in enumerate(ltiles):
        ps = pp.tile([P, 512], F32, tag="ps", name="ps_af")[:lsz, :100]
        _acc_mm(nc, ps, [(of[:, ki, loff:loff + lsz], fw2_sb[:, ki, :])
                         for ki in range(HT)])
        nc.vector.tensor_copy(out=aF[:lsz, lt, :], in_=ps)
    # GT [n'-part, m] = aF^T @ aT  -> gt_sb[:, :, e]
    ps = pp.tile([P, 512], F32, tag="ps", name="ps_gt")[:100, :100]
    _acc_mm(nc, ps, [(aF[:lsz, lt, :], aT[:lsz, lt, :])
                     for lt, (loff, lsz) in enumerate(ltiles)])
    nc.vector.tensor_copy(out=gt_sb[:100, :, e], in_=ps)


def _emit_ft(nc, pools, gt_sb, ftw_dram, out_sb, col0, engs=None):
    """out[:, col0:col0+1000] = vec(G)^T @ ft_w for all 8 elems at once.

    Weight tiles stream via all four engines' DMA queue sets — a single
    engine's 8 HWDGE queues cap at ~200 GB/s, which gates the matmuls."""
    pp_acc, ftp = pools["acc"], pools["ft"]
    ps = [pp_acc.tile([P, 512], F32, tag="acc", name="acc")[:BPC, :500]
          for _ in range(2)]
    for m in range(100):
        wt = ftp.tile([100, 1000], BF16, tag="ftw", name="ftw")
        (engs[m % len(engs)] if engs else nc.sync).dma_start(wt[:], ftw_dram[m])
        for ci, c0 in enumerate((0, 500)):
            nc.tensor.matmul(ps[ci], gt_sb[:100, m, :], wt[:, c0:c0 + 500],
                             start=(m == 0), stop=(m == 99))
    for ci, c0 in enumerate((0, 500)):
        nc.vector.tensor_copy(out=out_sb[:BPC, col0 + c0:col0 + c0 + 500],
                              in_=ps[ci])


def _build():
    nc = bacc.Bacc(None, target_bir_lowering=False)
    dt = lambda name, shape, dtype=F32R: nc.dram_tensor(name, shape, dtype,
                                                        kind="ExternalInput")
    xat = dt("xat", [BPC, LA, H], BF16)
    xaf = dt("xaf", [BPC, P, HT, LA], BF16)
    xbt = dt("xbt", [BPC, LB, H], BF16)
    xbf = dt("xbf", [BPC, P, HT, LB], BF16)
    aqkv = dt("aqkv", [2, P, HT, H], BF16)
    bqkv = dt("bqkv", [2, P, HT, H], BF16)
    afw = dt("afw", [2, LA, LA], BF16)
    bfw = dt("bfw", [2, LB, LB], BF16)
    atw = dt("atw", [P, HT, 100], BF16)
    afw2 = dt("afw2", [P, HT, 100], BF16)
    btw = dt("btw", [P, HT, 100], BF16)
    bfw2 = dt("bfw2", [P, HT, 100], BF16)
    aftw = dt("aftw", [100, 100, 1000], BF16)
    bftw = dt("bftw", [100, 100, 1000], BF16)
    hct = dt("hct", [P, 12, BPC])
    whead = dt("whead", [P, 12, 4])
    out = nc.dram_tensor("out", [BPC, 2003], F32, kind="ExternalOutput")

    with tile.TileContext(nc) as tc:
        import contextlib
        with contextlib.ExitStack() as ctx:
            singles = ctx.enter_context(tc.tile_pool(name="singles", bufs=1))
            acts = ctx.enter_context(tc.tile_pool(name="acts", bufs=2))
            work = ctx.enter_context(tc.tile_pool(name="work", bufs=2))
            wpool = ctx.enter_context(tc.tile_pool(name="wpool", bufs=3))
            spool = ctx.enter_context(tc.tile_pool(name="spool", bufs=1))
            ftp_a = ctx.enter_context(tc.tile_pool(name="ftp_a", bufs=8))
            ftp_b = ctx.enter_context(tc.tile_pool(name="ftp_b", bufs=24))
            pp = ctx.enter_context(tc.tile_pool(name="pp", bufs=6, space="PSUM"))
            pacc = ctx.enter_context(tc.tile_pool(name="pacc", bufs=2, space="PSUM"))

            ones = singles.tile([P, 1], BF16, name="ones")
            nc.vector.memset(ones, 1.0)
            out_sb = singles.tile([BPC, 2003], F32, name="out_sb")
            gta = singles.tile([100, 100, BPC], BF16, name="gta")
            gtb = singles.tile([100, 100, BPC], BF16, name="gtb")

            pools = {"work": work, "acts": acts, "ps": pp, "acc": pacc,
                     "ones": ones}

            def load_weights(qkv_dram, fw_dram, tw_dram, fw2_dram, L, ltiles):
                qkv_sb = []
                for j in range(2):
                    t = wpool.tile([P, 1, HT, H], BF16, tag="qkvj", name="qkvj")
                    for ki in range(HT):
                        nc.sync.dma_start(t[:, 0, ki], qkv_dram[j, :, ki])
                    qkv_sb.append(t)
                fw_sb = spool.tile([P, 2, 3, LA], BF16, tag="fw", name="fw")
                for j in range(2):
                    for ki, (koff, ksz) in enumerate(ltiles):
                        nc.sync.dma_start(fw_sb[:ksz, j, ki, :L],
                                          fw_dram[j, koff:koff + ksz, :])
                tw_sb = spool.tile([P, HT, 100], BF16, tag="tw", name="tw")
                nc.sync.dma_start(tw_sb[:], tw_dram.ap())
                fw2_sb = spool.tile([P, HT, 100], BF16, tag="fw2", name="fw2")
                nc.sync.dma_start(fw2_sb[:], fw2_dram.ap())
                return qkv_sb, fw_sb, tw_sb, fw2_sb

            class QKV:
                def __init__(self, tiles):
                    self.tiles = tiles

                def __getitem__(self, idx):
                    j, ki, sl = idx
                    return self.tiles[j][:, 0, ki, sl]

            # ---- branch a ----
            sca, scf_a = 1.0 / math.sqrt(H), 1.0 / math.sqrt(LA)
            scb, scf_b = 1.0 / math.sqrt(H), 1.0 / math.sqrt(LB)

            class _QkvAP:
                """qkv_sb[:, j, ki, sl] adapter over 3 per-j tiles."""
                def __init__(self, tiles):
                    self.t = tiles

                def __getitem__(self, key):
                    sl_p, j, ki, sl_f = key
                    return self.t[j][sl_p, 0, ki, sl_f]

            pre0 = _load_acts(nc, acts, 0, LA, LT_A, xat.ap(), xaf.ap())
            a_w = load_weights(aqkv, afw, atw, afw2, LA, LT_A)
            qkv_a_v = _QkvAP(a_w[0])

            # ---- head: predicts = hcat @ (fnn1 @ fnn2), folded on host ----
            hct_sb = singles.tile([P, 12, BPC], F32R, name="hct_sb")
            nc.sync.dma_start(hct_sb[:], hct.ap())
            whead_sb = singles.tile([P, 12, 4], F32R, name="whead_sb")
            nc.sync.dma_start(whead_sb[:], whead.ap())
            ps = pp.tile([P, 512], F32, tag="ps", name="ps_pred")[:BPC, :4]
            _acc_mm(nc, ps, [(hct_sb[:, ki, :], whead_sb[:, ki, :])
                             for ki in range(12)])
            nc.vector.tensor_copy(out=out_sb[:BPC, 0:3], in_=ps[:, :3])


            for e in range(BPC):
                _emit_branch(nc, pools, e, LA, LT_A, xat.ap(), xaf.ap(),
                             qkv_a_v, a_w[1], a_w[2], a_w[3], gta, sca, scf_a,
                             preloaded=pre0 if e == 0 else None)

            # branch-b weights prefetch + aft matmul overlap window
            b_w = load_weights(bqkv, bfw, btw, bfw2, LB, LT_B)
            qkv_b_v = _QkvAP(b_w[0])
            _emit_ft(nc, {**pools, "ft": ftp_a}, gta, aftw.ap(), out_sb, 3)

            # ---- branch b ----
            for e in range(BPC):
                _emit_branch(nc, pools, e, LB, LT_B, xbt.ap(), xbf.ap(),
                             qkv_b_v, b_w[1], b_w[2], b_w[3], gtb, scb, scf_b)
            _emit_ft(nc, {**pools, "ft": ftp_b}, gtb, bftw.ap(), out_sb, 1003,
                     engs=[nc.sync, nc.scalar])

            nc.sync.dma_start(out.ap()[:, :], out_sb[:BPC, :])
    nc.finalize()
    return nc


def _prep_inputs(tokens, cls_tokens, a_qkv_w, af_qkv_w, b_qkv_w, bf_qkv_w,
                 atW_w, afW_w, btW_w, bfW_w, aft_w, bft_w, fnn1_w, fnn2_w):
    """Host-side tiling into the DRAM layouts the kernel expects."""
    f32 = np.float32
    bft16 = ml_dtypes.bfloat16

    def ptile(w, dtype=bft16):  # [K, N] -> [P, K//P, N]
        K, N = w.shape
        return np.ascontiguousarray(
            w.reshape(K // P, P, N).transpose(1, 0, 2)).astype(dtype)

    aqw = np.asarray(a_qkv_w, f32)
    bqw = np.asarray(b_qkv_w, f32)
    afwf = np.asarray(af_qkv_w, f32)
    bfwf = np.asarray(bf_qkv_w, f32)
    common = {
        "aqkv": np.stack([ptile(aqw[0] @ aqw[1].T), ptile(aqw[2])]),
        "bqkv": np.stack([ptile(bqw[0] @ bqw[1].T), ptile(bqw[2])]),
        "afw": np.ascontiguousarray(
            np.stack([afwf[0] @ afwf[1].T, afwf[2]])).astype(bft16),
        "bfw": np.ascontiguousarray(
            np.stack([bfwf[0] @ bfwf[1].T, bfwf[2]])).astype(bft16),
        "atw": ptile(atW_w), "afw2": ptile(afW_w),
        "btw": ptile(btW_w), "bfw2": ptile(bfW_w),
        "aftw": np.ascontiguousarray(aft_w.reshape(100, 100, 1000)).astype(bft16),
        "bftw": np.ascontiguousarray(bft_w.reshape(100, 100, 1000)).astype(bft16),
        "whead": ptile(np.pad(np.asarray(fnn1_w, f32) @ np.asarray(fnn2_w, f32),
                              ((0, 0), (0, 1))), f32),
    }
    in_maps = []
    for c in range(NCORE):
        sl = slice(c * BPC, (c + 1) * BPC)
        xa = np.ascontiguousarray(tokens[sl, 1:, :]).astype(bft16)  # [8,272,768]
        xb = np.ascontiguousarray(cls_tokens[sl, 1:, :]).astype(bft16)
        xafm = np.ascontiguousarray(
            xa.transpose(0, 2, 1).reshape(BPC, HT, P, LA).transpose(0, 2, 1, 3))
        xbfm = np.ascontiguousarray(
            xb.transpose(0, 2, 1).reshape(BPC, HT, P, LB).transpose(0, 2, 1, 3))
        hcat = np.concatenate([tokens[sl, 0, :], cls_tokens[sl, 0, :]],
                              axis=1).astype(f32)                    # [8,1536]
        hctm = np.ascontiguousarray(
            hcat.T.reshape(12, P, BPC).transpose(1, 0, 2))
        in_maps.append({**common, "xat": xa, "xbt": xb, "xaf": xafm,
                        "xbf": xbfm, "hct": hctm})
    return in_maps


def kernel(**inputs):
    if "nc" not in _CACHE:
        _CACHE["nc"] = _build()
    nc = _CACHE["nc"]
    in_maps = _prep_inputs(
        inputs["tokens"], inputs["cls_tokens"], inputs["a_qkv_w"],
        inputs["af_qkv_w"], inputs["b_qkv_w"], inputs["bf_qkv_w"],
        inputs["atW_w"], inputs["afW_w"], inputs["btW_w"], inputs["bfW_w"],
        inputs["aft_w"], inputs["bft_w"], inputs["fnn1_w"], inputs["fnn2_w"])
    res = run_bass_kernel_spmd(nc, in_maps, core_ids=list(range(NCORE)),
                               **_CACHE.get("run_kwargs", {}))
    _CACHE["last_results"] = res
    return np.concatenate([res.results[c]["out"] for c in range(NCORE)],
                          axis=0).astype(np.float32)


# revision 18
# speedup vs baseline: 1.0011x; 1.0011x over previous
"""Trainium2 Bass kernel for nn_A_9835475108252 (dense_transformer).

Sharding: pure data parallel — batch 64 split as 8 elements per NeuronCore,
all weights replicated. Inside each core, per batch element:
  branch a/b = token-axis self-attention (TSA) + feature-axis SA (FSA),
  projections to 100 dims, G = aT^T@aF outer-product-ish [100,100],
  vec(G) @ {a,b}ft_w [10000,1000]; plus a tiny 2-layer head on CLS tokens.

Layout strategy: every matmul is expressed as out = lhsT^T @ rhs with the
contraction dim on partitions; the host uploads both [l,d] and [d,l] copies
of the activations so the device never transposes. Softmax denominators are
computed with PE matmuls against a ones vector, oriented so the 1/sum scale
folds into a per-partition tensor_scalar at the next eviction.
"""
import math
import numpy as np
import ml_dtypes

import concourse.bass as bass
from concourse import bacc
import concourse.mybir as mybir
import concourse.tile as tile
from concourse.bass_utils import run_bass_kernel_spmd

P = 128
F32 = mybir.dt.float32
F32R = mybir.dt.float32r
BF16 = mybir.dt.bfloat16
EXP = mybir.ActivationFunctionType.Exp

B, NCORE, BPC, H = 64, 8, 8, 768
LA, LB = 272, 256
HT = H // P  # 6
LT_A = [(0, 128), (128, 128), (256, 16)]
LT_B = [(0, 128), (128, 128)]

_CACHE: dict = {}


def _acc_mm(nc, ps, pairs):
    n = len(pairs)
    for i, (lt, rt) in enumerate(pairs):
        nc.tensor.matmul(ps, lt, rt, start=(i == 0), stop=(i == n - 1))


def _load_acts(nc, acts, e, L, ltiles, xt_dram, xf_dram):
    xt = acts.tile([P, 3, H], BF16, tag="xt", name="xt")  # [l-part, lo, d]
    for i, (off, sz) in enumerate(ltiles):
        nc.sync.dma_start(xt[:sz, i, :], xt_dram[e, off:off + sz, :])
    xf = acts.tile([P, HT, LA], BF16, tag="xf", name="xf")  # [d-part, do, l]
    nc.sync.dma_start(xf[:, :, :L], xf_dram[e])
    return xt, xf


def _emit_branch(nc, pools, e, L, ltiles, xt_dram, xf_dram, qkv_sb, fw_sb,
                 tw_sb, fw2_sb, gt_sb, scale_t, scale_f, preloaded=None):
    work, acts, pp = pools["work"], pools["acts"], pools["ps"]
    ones = pools["ones"]

    # ---- activations -------------------------------------------------
    xt, xf = preloaded if preloaded is not None else _load_acts(
        nc, acts, e, L, ltiles, xt_dram, xf_dram)

    # ---- TSA ----------------------------------------------------------
    # uf [d_out-part, do, l] = (x @ (Wq Wk^T))^T  (Q/K folded on host)
    uf = work.tile([P, HT, LA], BF16, tag="qf", name="uf")
    for do in range(HT):
        ps = pp.tile([P, 512], F32, tag="ps", name="ps_q")[:, :L]
        _acc_mm(nc, ps, [(qkv_sb[:, 0, ki, do * P:(do + 1) * P],
                          xf[:, ki, :L]) for ki in range(HT)])
        nc.vector.tensor_copy(out=uf[:, do, :L], in_=ps)
    # v [l-part, lo, d] = x @ Wv
    v = work.tile([P, 3, H], BF16, tag="v", name="v")
    for lt, (off, sz) in enumerate(ltiles):
        for c0 in (0, 384):
            ps = pp.tile([P, 512], F32, tag="ps", name="ps_v")[:sz, :384]
            _acc_mm(nc, ps, [(xf[:, ki, off:off + sz],
                              qkv_sb[:, 1, ki, c0:c0 + 384]) for ki in range(HT)])
            nc.vector.tensor_copy(out=v[:sz, lt, c0:c0 + 384], in_=ps)
    # est [m-part, mo, l] = exp(scale * S^T),  S = q @ k^T
    est = work.tile([P, 3, LA], BF16, tag="est", name="est")
    for mt, (moff, msz) in enumerate(ltiles):
        ps = pp.tile([P, 512], F32, tag="ps", name="ps_st")[:msz, :L]
        _acc_mm(nc, ps, [(xf[:, ki, moff:moff + msz], uf[:, ki, :L])
                         for ki in range(HT)])
        nc.scalar.activation(out=est[:msz, mt, :L], in_=ps, func=EXP,
                             scale=scale_t)
    # sinv [l-part, lo, 1] = 1 / colsum(est)
    sinv = work.tile([P, 3, 1], F32, tag="sinv", name="sinv")
    for lt, (loff, lsz) in enumerate(ltiles):
        ps = pp.tile([P, 512], F32, tag="ps", name="ps_si")[:lsz, :1]
        _acc_mm(nc, ps, [(est[:msz, mt, loff:loff + lsz], ones[:msz, :])
                         for mt, (moff, msz) in enumerate(ltiles)])
        nc.vector.reciprocal(out=sinv[:lsz, lt, :], in_=ps)
    # oa [d-part, do, l] = v^T @ est   (unnormalized attention out, F-layout)
    oa = work.tile([P, HT, LA], BF16, tag="oa", name="oa")
    for do in range(HT):
        ps = pp.tile([P, 512], F32, tag="ps", name="ps_oa")[:, :L]
        _acc_mm(nc, ps, [(v[:msz, mt, do * P:(do + 1) * P], est[:msz, mt, :L])
                         for mt, (moff, msz) in enumerate(ltiles)])
        nc.vector.tensor_copy(out=oa[:, do, :L], in_=ps)
    # aT [l-part, lo, 100] = (aTSA @ atW) * sinv  (softmax norm folded here)
    aT = work.tile([P, 3, 100], BF16, tag="aT", name="aT")
    for lt, (loff, lsz) in enumerate(ltiles):
        ps = pp.tile([P, 512], F32, tag="ps", name="ps_at")[:lsz, :100]
        _acc_mm(nc, ps, [(oa[:, ki, loff:loff + lsz], tw_sb[:, ki, :])
                         for ki in range(HT)])
        nc.vector.tensor_scalar_mul(out=aT[:lsz, lt, :], in0=ps,
                                    scalar1=sinv[:lsz, lt, :])

    # ---- FSA ----------------------------------------------------------
    # wt [t-part, to, d] = (Wfq Wfk^T)^T @ x   (Q/K folded on host)
    wt = work.tile([P, 3, H], BF16, tag="qft", name="wt")
    for tt, (toff, tsz) in enumerate(ltiles):
        for c0 in (0, 384):
            ps = pp.tile([P, 512], F32, tag="ps", name="ps_qf")[:tsz, :384]
            _acc_mm(nc, ps, [(fw_sb[:ksz, 0, ki, toff:toff + tsz],
                              xt[:ksz, ki, c0:c0 + 384])
                             for ki, (koff, ksz) in enumerate(ltiles)])
            nc.vector.tensor_copy(out=wt[:tsz, tt, c0:c0 + 384], in_=ps)
    # vf [e-part, eo, t (+ ones col at L)] = x^T @ Wfv
    vf = work.tile([P, HT, LA + 1], BF16, tag="vf", name="vf")
    nc.vector.memset(vf[:, :, L:L + 1], 1.0)
    for eo in range(HT):
        ps = pp.tile([P, 512], F32, tag="ps", name="ps_vf")[:, :L]
        _acc_mm(nc, ps, [(xt[:ksz, ki, eo * P:(eo + 1) * P],
                          fw_sb[:ksz, 1, ki, :L])
                         for ki, (koff, ksz) in enumerate(ltiles)])
        nc.vector.tensor_copy(out=vf[:, eo, :L], in_=ps)
    # esf [e-part, eo, d] = exp(scale * Sf^T)
    esf = work.tile([P, HT, H], BF16, tag="esf", name="esf")
    for eo in range(HT):
        for c0 in (0, 384):
            ps = pp.tile([P, 512], F32, tag="ps", name="ps_sf")[:, :384]
            _acc_mm(nc, ps, [(xt[:tsz, ki, eo * P:(eo + 1) * P],
                              wt[:tsz, ki, c0:c0 + 384])
                             for ki, (toff, tsz) in enumerate(ltiles)])
            nc.scalar.activation(out=esf[:, eo, c0:c0 + 384], in_=ps, func=EXP,
                                 scale=scale_f)
    # of [d-part, do, t] = (Pf @ vf); the vf ones-column makes psum col L
    # hold colsum(esf) = the softmax denominator, normalized at eviction
    sfinv = work.tile([P, HT, 1], F32, tag="sfinv", name="sfinv")
    of = work.tile([P, HT, LA], BF16, tag="of", name="of")
    for do in range(HT):
        ps = pp.tile([P, 512], F32, tag="ps", name="ps_of")[:, :L + 1]
        _acc_mm(nc, ps, [(esf[:, ki, do * P:(do + 1) * P], vf[:, ki, :L + 1])
                         for ki in range(HT)])
        nc.vector.reciprocal(out=sfinv[:, do, :], in_=ps[:, L:L + 1])
        nc.vector.tensor_scalar_mul(out=of[:, do, :L], in0=ps[:, :L],
                                    scalar1=sfinv[:, do, :])
    # aF [l-part, lo, 100] = aFSA @ afW
    aF = work.tile([P, 3, 100], BF16, tag="aF", name="aF")
    for lt, (loff, lsz) in enumerate(ltiles):
        ps = pp.tile([P, 512], F32, tag="ps", name="ps_af")[:lsz, :100]
        _acc_mm(nc, ps, [(of[:, ki, loff:loff + lsz], fw2_sb[:, ki, :])
                         for ki in range(HT)])
        nc.vector.tensor_copy(out=aF[:lsz, lt, :], in_=ps)
    # GT [n'-part, m] = aF^T @ aT  -> gt_sb[:, :, e]
    ps = pp.tile([P, 512], F32, tag="ps", name="ps_gt")[:100, :100]
    _acc_mm(nc, ps, [(aF[:lsz, lt, :], aT[:lsz, lt, :])
                     for lt, (loff, lsz) in enumerate(ltiles)])
    nc.vector.tensor_copy(out=gt_sb[:100, :, e], in_=ps)


def _emit_ft(nc, pools, gt_sb, ftw_dram, out_sb, col0, engs=None):
    """out[:, col0:col0+1000] = vec(G)^T @ ft_w for all 8 elems at once.

    For the final (non-overlapped) stream, weight tiles alternate between
    engine DMA queue sets — one engine's 8 HWDGE queues cap at ~200 GB/s."""
    pp_acc, ftp = pools["acc"], pools["ft"]
    ps = [pp_acc.tile([P, 512], F32, tag="acc", name="acc")[:BPC, :500]
          for _ in range(2)]
    for m in range(100):
        wt = ftp.tile([100, 1000], BF16, tag="ftw", name="ftw")
        (engs[m % len(engs)] if engs else nc.sync).dma_start(wt[:], ftw_dram[m])
        for ci, c0 in enumerate((0, 500)):
            nc.tensor.matmul(ps[ci], gt_sb[:100, m, :], wt[:, c0:c0 + 500],
                             start=(m == 0), stop=(m == 99))
    for ci, c0 in enumerate((0, 500)):
        nc.vector.tensor_copy(out=out_sb[:BPC, col0 + c0:col0 + c0 + 500],
                              in_=ps[ci])


def _build():
    nc = bacc.Bacc(None, target_bir_lowering=False)
    dt = lambda name, shape, dtype=F32R: nc.dram_tensor(name, shape, dtype,
                                                        kind="ExternalInput")
    xat = dt("xat", [BPC, LA, H], BF16)
    xaf = dt("xaf", [BPC, P, HT, LA], BF16)
    xbt = dt("xbt", [BPC, LB, H], BF16)
    xbf = dt("xbf", [BPC, P, HT, LB], BF16)
    aqkv = dt("aqkv", [2, P, HT, H], BF16)
    bqkv = dt("bqkv", [2, P, HT, H], BF16)
    afw = dt("afw", [2, LA, LA], BF16)
    bfw = dt("bfw", [2, LB, LB], BF16)
    atw = dt("atw", [P, HT, 100], BF16)
    afw2 = dt("afw2", [P, HT, 100], BF16)
    btw = dt("btw", [P, HT, 100], BF16)
    bfw2 = dt("bfw2", [P, HT, 100], BF16)
    aftw = dt("aftw", [100, 100, 1000], BF16)
    bftw = dt("bftw", [100, 100, 1000], BF16)
    hct = dt("hct", [P, 12, BPC])
    whead = dt("whead", [P, 12, 4])
    out = nc.dram_tensor("out", [BPC, 2003], F32, kind="ExternalOutput")

    with tile.TileContext(nc) as tc:
        import contextlib
        with contextlib.ExitStack() as ctx:
            singles = ctx.enter_context(tc.tile_pool(name="singles", bufs=1))
            acts = ctx.enter_context(tc.tile_pool(name="acts", bufs=2))
            work = ctx.enter_context(tc.tile_pool(name="work", bufs=2))
            wpool = ctx.enter_context(tc.tile_pool(name="wpool", bufs=3))
            spool = ctx.enter_context(tc.tile_pool(name="spool", bufs=1))
            ftp_a = ctx.enter_context(tc.tile_pool(name="ftp_a", bufs=8))
            ftp_b = ctx.enter_context(tc.tile_pool(name="ftp_b", bufs=24))
            pp = ctx.enter_context(tc.tile_pool(name="pp", bufs=6, space="PSUM"))
            pacc = ctx.enter_context(tc.tile_pool(name="pacc", bufs=2, space="PSUM"))

            ones = singles.tile([P, 1], BF16, name="ones")
            nc.vector.memset(ones, 1.0)
            out_sb = singles.tile([BPC, 2003], F32, name="out_sb")
            gta = singles.tile([100, 100, BPC], BF16, name="gta")
            gtb = singles.tile([100, 100, BPC], BF16, name="gtb")

            pools = {"work": work, "acts": acts, "ps": pp, "acc": pacc,
                     "ones": ones}

            def load_weights(qkv_dram, fw_dram, tw_dram, fw2_dram, L, ltiles):
                qkv_sb = []
                for j in range(2):
                    t = wpool.tile([P, 1, HT, H], BF16, tag="qkvj", name="qkvj")
                    for ki in range(HT):
                        nc.sync.dma_start(t[:, 0, ki], qkv_dram[j, :, ki])
                    qkv_sb.append(t)
                fw_sb = spool.tile([P, 2, 3, LA], BF16, tag="fw", name="fw")
                for j in range(2):
                    for ki, (koff, ksz) in enumerate(ltiles):
                        nc.sync.dma_start(fw_sb[:ksz, j, ki, :L],
                                          fw_dram[j, koff:koff + ksz, :])
                tw_sb = spool.tile([P, HT, 100], BF16, tag="tw", name="tw")
                nc.sync.dma_start(tw_sb[:], tw_dram.ap())
                fw2_sb = spool.tile([P, HT, 100], BF16, tag="fw2", name="fw2")
                nc.sync.dma_start(fw2_sb[:], fw2_dram.ap())
                return qkv_sb, fw_sb, tw_sb, fw2_sb

            # ---- branch a ----
            sca, scf_a = 1.0 / math.sqrt(H), 1.0 / math.sqrt(LA)
            scb, scf_b = 1.0 / math.sqrt(H), 1.0 / math.sqrt(LB)

            class _QkvAP:
                """qkv_sb[:, j, ki, sl] adapter over per-j weight tiles."""
                def __init__(self, tiles):
                    self.t = tiles

                def __getitem__(self, key):
                    sl_p, j, ki, sl_f = key
                    return self.t[j][sl_p, 0, ki, sl_f]

            pre0 = _load_acts(nc, acts, 0, LA, LT_A, xat.ap(), xaf.ap())
            a_w = load_weights(aqkv, afw, atw, afw2, LA, LT_A)
            qkv_a_v = _QkvAP(a_w[0])

            # ---- head: predicts = hcat @ (fnn1 @ fnn2), folded on host ----
            hct_sb = singles.tile([P, 12, BPC], F32R, name="hct_sb")
            nc.sync.dma_start(hct_sb[:], hct.ap())
            whead_sb = singles.tile([P, 12, 4], F32R, name="whead_sb")
            nc.sync.dma_start(whead_sb[:], whead.ap())
            ps = pp.tile([P, 512], F32, tag="ps", name="ps_pred")[:BPC, :4]
            _acc_mm(nc, ps, [(hct_sb[:, ki, :], whead_sb[:, ki, :])
                             for ki in range(12)])
            nc.vector.tensor_copy(out=out_sb[:BPC, 0:3], in_=ps[:, :3])


            for e in range(BPC):
                _emit_branch(nc, pools, e, LA, LT_A, xat.ap(), xaf.ap(),
                             qkv_a_v, a_w[1], a_w[2], a_w[3], gta, sca, scf_a,
                             preloaded=pre0 if e == 0 else None)

            # branch-b weights prefetch + aft matmul overlap window
            b_w = load_weights(bqkv, bfw, btw, bfw2, LB, LT_B)
            qkv_b_v = _QkvAP(b_w[0])
            _emit_ft(nc, {**pools, "ft": ftp_a}, gta, aftw.ap(), out_sb, 3)

            # ---- branch b ----
            for e in range(BPC):
                _emit_branch(nc, pools, e, LB, LT_B, xbt.ap(), xbf.ap(),
                             qkv_b_v, b_w[1], b_w[2], b_w[3], gtb, scb, scf_b)
            _emit_ft(nc, {**pools, "ft": ftp_b}, gtb, bftw.ap(), out_sb, 1003,
                     engs=[nc.sync, nc.scalar])

            nc.sync.dma_start(out.ap()[:, :], out_sb[:BPC, :])
    nc.finalize()
    return nc


def _prep_inputs(tokens, cls_tokens, a_qkv_w, af_qkv_w, b_qkv_w, bf_qkv_w,
                 atW_w, afW_w, btW_w, bfW_w, aft_w, bft_w, fnn1_w, fnn2_w):
    """Host-side tiling into the DRAM layouts the kernel expects."""
    f32 = np.float32
    bft16 = ml_dtypes.bfloat16

    def ptile(w, dtype=bft16):  # [K, N] -> [P, K//P, N]
        K, N = w.shape
        return np.ascontiguousarray(
            w.reshape(K // P, P, N).transpose(1, 0, 2)).astype(dtype)

    aqw = np.asarray(a_qkv_w, f32)
    bqw = np.asarray(b_qkv_w, f32)
    afwf = np.asarray(af_qkv_w, f32)
    bfwf = np.asarray(bf_qkv_w, f32)
    common = {
        "aqkv": np.stack([ptile(aqw[0] @ aqw[1].T), ptile(aqw[2])]),
        "bqkv": np.stack([ptile(bqw[0] @ bqw[1].T), ptile(bqw[2])]),
        "afw": np.ascontiguousarray(
            np.stack([afwf[0] @ afwf[1].T, afwf[2]])).astype(bft16),
        "bfw": np.ascontiguousarray(
            np.stack([bfwf[0] @ bfwf[1].T, bfwf[2]])).astype(bft16),
        "atw": ptile(atW_w), "afw2": ptile(afW_w),
        "btw": ptile(btW_w), "bfw2": ptile(bfW_w),
        "aftw": np.ascontiguousarray(aft_w.reshape(100, 100, 1000)).astype(bft16),
        "bftw": np.ascontiguousarray(bft_w.reshape(100, 100, 1000)).astype(bft16),
        "whead": ptile(np.pad(np.asarray(fnn1_w, f32) @ np.asarray(fnn2_w, f32),
                              ((0, 0), (0, 1))), f32),
    }
    in_maps = []
    for c in range(NCORE):
        sl = slice(c * BPC, (c + 1) * BPC)
        xa = np.ascontiguousarray(tokens[sl, 1:, :]).astype(bft16)  # [8,272,768]
        xb = np.ascontiguousarray(cls_tokens[sl, 1:, :]).astype(bft16)
        xafm = np.ascontiguousarray(
            xa.transpose(0, 2, 1).reshape(BPC, HT, P, LA).transpose(0, 2, 1, 3))
        xbfm = np.ascontiguousarray(
            xb.transpose(0, 2, 1).reshape(BPC, HT, P, LB).transpose(0, 2, 1, 3))
        hcat = np.concatenate([tokens[sl, 0, :], cls_tokens[sl, 0, :]],
                              axis=1).astype(f32)                    # [8,1536]
        hctm = np.ascontiguousarray(
            hcat.T.reshape(12, P, BPC).transpose(1, 0, 2))
        in_maps.append({**common, "xat": xa, "xbt": xb, "xaf": xafm,
                        "xbf": xbfm, "hct": hctm})
    return in_maps


def kernel(**inputs):
    if "nc" not in _CACHE:
        _CACHE["nc"] = _build()
    nc = _CACHE["nc"]
    in_maps = _prep_inputs(
        inputs["tokens"], inputs["cls_tokens"], inputs["a_qkv_w"],
        inputs["af_qkv_w"], inputs["b_qkv_w"], inputs["bf_qkv_w"],
        inputs["atW_w"], inputs["afW_w"], inputs["btW_w"], inputs["bfW_w"],
        inputs["aft_w"], inputs["bft_w"], inputs["fnn1_w"], inputs["fnn2_w"])
    res = run_bass_kernel_spmd(nc, in_maps, core_ids=list(range(NCORE)),
                               **_CACHE.get("run_kwargs", {}))
    _CACHE["last_results"] = res
    return np.concatenate([res.results[c]["out"] for c in range(NCORE)],
                          axis=0).astype(np.float32)
